# revision 63
# baseline (speedup 1.0000x reference)
"""Additive (Bahdanau) attention on 8 TRN2 NeuronCores.

Problem: B=8, LQ=256, LK=1024, DQ=DK=DV=512, H=128.
  q = Q @ W_q; k = K @ W_k
  scores[b,q,k] = sum_h w_v[h] * tanh(qf[b,q,h] + kf[b,k,h])
  out = softmax_k(mask(scores)) @ V

Sharding: data-parallel over QUERIES - core c computes query rows
[32c, 32c+32) of every batch; per-core work is identical, no cross-core
communication. The graph is compiled for the actual valid_lengths: the
tanh/score/softmax path runs at the exact valid length.

Column-scores pipeline (h=H=128 on SBUF partitions):
  - X[h, q, k] = kf + qf built by DVE tensor_scalar adds (f16, 4x mode);
    tanh runs in-place on ACT over multi-query groups (one 222-cycle
    init amortized over 8 queries instead of per query).
  - Scores via per-(query, key-chunk) matmuls with the tanh tile as the
    PE stationary and w_v as the single streamed column: out free size
    is 1, so each costs ~a cycle; scores land (keys on partitions,
    queries on free) in one small PSUM tile per batch. No stg copies,
    no gather DMAs, no eT transposes.
  - Ragged tail keys are pre-filled with -50 by one ones@(-50/128)
    matmul (base partition 32-aligned; valid rows overwritten by the
    real score matmuls), so exp underflows to 0 there.
  - One exp per batch reads scores straight from PSUM at full 128-
    partition utilization; e (f16, SBUF) is directly the attn@V lhsT.
    Row sums come from an extra ones-column matmul into PSUM; the
    output is scaled by the DVE reciprocal of that column.
  - A fraction of queries evaluates tanh as a clamped degree-9 odd
    polynomial on DVE/Pool instead (only the first fused add+clamp
    step needs DVE's AP-scalar form; the other 10 steps are balanced
    between DVE (2x/4x f16 modes) and Pool by a greedy that tracks
    projected engine busy).
  - PSUM-to-SBUF copies (kT, kf, qT) are likewise balanced DVE/Pool.

ACT is still the roofline but now near its 0.833 ns/col floor; the
planner picks the poly offload so ACT ~= DVE ~= Pool.
"""

import sys

if "/opt/trn_rl_repo" not in sys.path:
    sys.path.insert(0, "/opt/trn_rl_repo")

import numpy as np
import ml_dtypes

import concourse.mybir as mybir
from concourse import tile, bacc
from concourse.bass_utils import run_bass_kernel_spmd

B, LQ, LK, DQ, DK, DV, H = 8, 256, 1024, 512, 512, 512, 128
N_CORES = 8
QPC = LQ // N_CORES  # 32 query rows per core per batch
NEG = -50.0

_F16 = mybir.dt.float16
_F32 = mybir.dt.float32

# Degree-9 odd minimax polynomial for tanh on [-3.6, 3.6], inputs clamped
# (max err ~0.012). Used for queries offloaded from ScalarE to DVE/Pool.
_C = 3.6
_PC = (0.95400865, -0.21577773, 0.03285149, -0.00246163, 6.961e-05)

_G = 8  # queries per batched-tanh ACT instruction

_cached = {}


class _Bal:
    """Greedy two-engine balancer: assign each flexible op to DVE or Pool,
    tracking projected busy-ns; mandatory work just accumulates."""

    def __init__(self):
        self.dve = 0.0
        self.pool = 0.0

    def add_dve(self, ns):
        self.dve += ns

    def add_pool(self, ns):
        self.pool += ns

    def pick(self, ns_dve, ns_pool):
        if self.dve + ns_dve <= self.pool + ns_pool:
            self.dve += ns_dve
            return "dve"
        self.pool += ns_pool
        return "pool"


def _ts4(n):  # DVE tensor_scalar f16 (4x)
    return (n / 4.0 + 58.0) / 0.96


def _tt2(n):  # DVE tensor_tensor f16 (2x)
    return (n / 2.0 + 58.0) / 0.96


def _cp2p(n):  # DVE copy f16 PSUM->SBUF (2x, psum init)
    return (n / 2.0 + 120.0) / 0.96


def _cp1p(n):  # DVE copy f32-src PSUM->SBUF (1x)
    return (n + 120.0) / 0.96


def _pool(n):  # Pool any elementwise op (calibrated vs CoreSim trace)
    return (n / 1.2) * 1.05 + 80.0


def _sim_totals(lens, y_bs):
    """Replay the per-engine cost sequence for a candidate poly plan."""
    bal = _Bal()
    act = 1283.0  # act table load
    bal.add_dve(_cp2p(512) * 2 + _cp1p(256))  # qT + qfT copies
    bal.add_dve(8 * 130.0)  # reciprocals
    for b, ln in enumerate(lens):
        ext = max(128, -(-ln // 128) * 128)
        nkc = ext // 128
        y = y_bs[b]
        for _ in range(nkc):  # kT copies
            bal.add_dve(_cp2p(512))
        for c0 in range(0, ln, 512):  # kf copies
            cn = min(512, ln - c0)
            bal.add_dve(_cp1p(cn))
        na = QPC - y
        bal.add_dve(na * _ts4(ln))  # adds
        ng = -(-na // _G) if na else 0
        act += (na * ln + 222 * ng) / 1.2  # batched tanh
        act += (nkc * QPC + 172) / 1.2  # exp
        for _ in range(y):  # poly queries
            bal.add_dve(_ts4(ln))  # s1 fused add+clamp
            for _ in range(5):
                bal.pick(_ts4(ln), _pool(ln))
            for _ in range(5):
                bal.pick(_tt2(ln), _pool(ln))
        bal.add_dve(_cp1p(512))  # out scale
    return act, bal.dve, bal.pool


def _batch_order(lens):
    """2nd-smallest first (short pipeline fill), smallest last (short drain)."""
    asc = sorted(range(len(lens)), key=lambda b: (lens[b], b))
    return [asc[1]] + asc[2:] + [asc[0]]


def _batch_cap(ln, nxt_ln, slack=500.0, maxy=5):
    """Largest poly count whose helper work still fits inside this batch's
    ACT window (poly beyond that bunches on DVE/Pool and stalls the next
    batch's adds)."""
    ext = max(128, -(-ln // 128) * 128)
    nkc = ext // 128
    nxt_ext = max(128, -(-nxt_ln // 128) * 128) if nxt_ln else 0
    best = 0
    for y in range(0, maxy + 1):
        na = QPC - y
        act = (na * ln + 222 * -(-na // _G) + nkc * QPC + 172) / 1.2
        dve = na * _ts4(ln) + y * _ts4(ln)
        if nxt_ln:
            dve += (nxt_ext // 128) * _cp2p(512)
            for c0 in range(0, nxt_ln, 512):
                dve += _cp1p(min(512, nxt_ln - c0))
        pool = 0.0
        for _ in range(y):
            for cd, cp in [(_ts4(ln), _pool(ln))] * 5 + [(_tt2(ln), _pool(ln))] * 5:
                if dve + cd <= pool + cp:
                    dve += cd
                else:
                    pool += cp
        if max(dve, pool) <= act + slack:
            best = y
    return best


def _plan(lens):
    """Pick per-batch poly-query counts minimizing max engine busy, subject
    to per-batch feasibility (helper work must overlap that batch's tanh)
    and no poly on the last batch (its chains would drain after ACT)."""
    lens = [int(l) for l in lens]
    bo = _batch_order(lens)
    cap = {}
    for i, b in enumerate(bo):
        nxt = lens[bo[i + 1]] if i + 1 < len(bo) else 0
        cap[b] = _batch_cap(lens[b], nxt)
    cap[bo[-1]] = 0
    best = None
    order = sorted(range(len(lens)), key=lambda b: -lens[b])
    maxy = sum(cap.values())
    for y_tot in range(0, maxy + 1):
        y_bs = [0] * len(lens)
        rem = y_tot
        while rem > 0:
            prog = False
            for b in order:
                if rem > 0 and y_bs[b] < cap[b]:
                    y_bs[b] += 1
                    rem -= 1
                    prog = True
            if not prog:
                break
        a, d, p = _sim_totals(lens, y_bs)
        t = max(a, d, p)
        if best is None or t < best[0]:
            best = (t, tuple(y_bs))
    return list(best[1])


def _build(lens):
    nc = bacc.Bacc("TRN2", target_bir_lowering=False, debug=False)
    AL = mybir.AluOpType
    AF = mybir.ActivationFunctionType

    lens = [int(l) for l in lens]
    extents = [max(128, ((l + 127) // 128) * 128) for l in lens]
    nkcs = [e // 128 for e in extents]
    offs = np.concatenate([[0], np.cumsum(extents)]).astype(int)
    total_k = int(sum(extents))
    y_bs = _plan(lens)

    Qp = nc.declare_dram_parameter("Q", [B * QPC, DQ], _F16, isOutput=False)
    Kp = nc.declare_dram_parameter("K", [total_k, DK], _F16, isOutput=False)
    Vp = nc.declare_dram_parameter("V", [total_k, DV], _F16, isOutput=False)
    Wqp = nc.declare_dram_parameter("Wq", [DQ, H], _F16, isOutput=False)
    Wkp = nc.declare_dram_parameter("Wk", [DK, H], _F16, isOutput=False)
    wvp = nc.declare_dram_parameter("wv", [H, 1], _F16, isOutput=False)
    idp = nc.declare_dram_parameter("ident", [128, 128], _F16, isOutput=False)
    outp = nc.declare_dram_parameter("out", [B, QPC, DV], _F32, isOutput=True)

    NDQ = DQ // 128
    bal = _Bal()
    bal.add_dve(8 * 130.0)

    with tile.TileContext(nc) as tc:
        with (
            tc.tile_pool(name="const", bufs=1) as const,
            tc.tile_pool(name="nat", bufs=4) as nat,
            tc.tile_pool(name="kv", bufs=3) as kv,
            tc.tile_pool(name="xg", bufs=5) as xg,
            tc.tile_pool(name="tpoly", bufs=20) as tpoly,
            tc.tile_pool(name="epool", bufs=2) as epool,
            tc.tile_pool(name="opool", bufs=2) as opool,
            tc.tile_pool(name="ps_s", bufs=2, space="PSUM") as ps_s,
            tc.tile_pool(name="ps_kp", bufs=2, space="PSUM") as ps_kp,
            tc.tile_pool(name="ps_tail", bufs=2, space="PSUM") as ps_tail,
        ):
            # ---- constants / weights -------------------------------------
            wq_sb = const.tile([128, NDQ, H], _F16)
            nc.gpsimd.dma_start(out=wq_sb, in_=Wqp[:, :].rearrange("(c p) h -> p c h", p=128))
            wk_sb = const.tile([128, NDQ, H], _F16)
            nc.gpsimd.dma_start(out=wk_sb, in_=Wkp[:, :].rearrange("(c p) h -> p c h", p=128))
            wv_sb = const.tile([H, 1], _F16)
            nc.gpsimd.dma_start(out=wv_sb, in_=wvp[:, :])
            ident = const.tile([128, 128], _F16)
            nc.sync.dma_start(out=ident, in_=idp[:, :])
            ones = const.tile([128, 128], _F16)
            nc.gpsimd.memset(ones, 1.0)
            negq = const.tile([128, QPC], _F16)
            nc.gpsimd.memset(negq, NEG / 128.0)
            onecol = const.tile([128, 1], _F16)
            nc.gpsimd.memset(onecol, 1.0)

            # ---- qfT (h, B*QPC) for this core's queries ------------------
            # (emitted AFTER the first batch's K DMAs: the kf chain is the
            # longer startup-critical path, so K goes first on the queues)
            qfT_sb = None

            def q_path():
                nonlocal qfT_sb
                qT_sb = const.tile([128, NDQ, B * QPC], _F16)
                for qt in range(B * QPC // 128):
                    qn = nat.tile([128, DQ], _F16, tag="nat")
                    nc.sync.dma_start(out=qn, in_=Qp[qt * 128 : (qt + 1) * 128, :])
                    pst = ps_kp.tile([128, 512], _F16, tag="kp")
                    for dc in range(NDQ):
                        nc.tensor.transpose(pst[:, dc * 128 : (dc + 1) * 128], qn[:, dc * 128 : (dc + 1) * 128], ident)
                    nc.vector.tensor_copy(
                        qT_sb[:, :, qt * 128 : (qt + 1) * 128],
                        pst.rearrange("p (c x) -> p c x", c=NDQ),
                    )
                bal.add_dve(_cp2p(512) * 2)
                qf_ps = ps_tail.tile([128, B * QPC], _F32, tag="tail")
                for dc in range(NDQ):
                    nc.tensor.matmul(
                        out=qf_ps,
                        lhsT=wq_sb[:, dc, :],
                        rhs=qT_sb[:, dc, :],
                        start=(dc == 0),
                        stop=(dc == NDQ - 1),
                    )
                qfT_sb = const.tile([128, B * QPC], _F32, name="qfT_sb")
                nc.vector.tensor_copy(qfT_sb, qf_ps)
                bal.add_dve(_cp1p(256))

            def flex_copy(out_ap, in_ap, n, f32src):
                # PSUM is unreachable from Pool: all PSUM->SBUF copies on DVE
                bal.add_dve(_cp1p(n) if f32src else _cp2p(n))
                nc.vector.tensor_copy(out_ap, in_ap)

            def flex_ts(out_ap, in_ap, s1, s2, op0, op1, n):
                eng = bal.pick(_ts4(n), _pool(n))
                e = nc.vector if eng == "dve" else nc.gpsimd
                if op1 is None:
                    e.tensor_scalar(out=out_ap, in0=in_ap, scalar1=s1, scalar2=None, op0=op0)
                else:
                    e.tensor_scalar(out=out_ap, in0=in_ap, scalar1=s1, scalar2=s2, op0=op0, op1=op1)

            def flex_tt(out_ap, a_ap, b_ap, n):
                eng = bal.pick(_tt2(n), _pool(n))
                e = nc.vector if eng == "dve" else nc.gpsimd
                e.tensor_tensor(out=out_ap, in0=a_ap, in1=b_ap, op=AL.mult)

            # ---- helpers --------------------------------------------------
            def k_path(b):
                ext, nkc, ln = extents[b], nkcs[b], lens[b]
                o0 = int(offs[b])
                kT_b = kv.tile([128, NDQ, ext], _F16, tag="kT", bufs=2)
                for kc in range(nkc):
                    kn = nat.tile([128, DK], _F16, tag="nat")
                    nc.sync.dma_start(out=kn, in_=Kp[o0 + kc * 128 : o0 + (kc + 1) * 128, :])
                    pst = ps_kp.tile([128, 512], _F16, tag="kp")
                    for dc in range(NDQ):
                        nc.tensor.transpose(pst[:, dc * 128 : (dc + 1) * 128], kn[:, dc * 128 : (dc + 1) * 128], ident)
                    flex_copy(
                        kT_b[:, :, kc * 128 : (kc + 1) * 128],
                        pst.rearrange("p (c x) -> p c x", c=NDQ),
                        512,
                        False,
                    )
                kf_sb = kv.tile([128, ln], _F16, tag="kf")
                for c0 in range(0, ln, 512):
                    cn = min(512, ln - c0)
                    kf_ps = ps_kp.tile([128, 512], _F32, tag="kp")
                    for dc in range(NDQ):
                        nc.tensor.matmul(
                            out=kf_ps[:, 0:cn],
                            lhsT=wk_sb[:, dc, :],
                            rhs=kT_b[:, dc, c0 : c0 + cn],
                            start=(dc == 0),
                            stop=(dc == NDQ - 1),
                        )
                    flex_copy(kf_sb[:, c0 : c0 + cn], kf_ps[:, 0:cn], cn, True)
                v_b = kv.tile([128, nkc, DV], _F16, tag="v")
                nc.gpsimd.dma_start(
                    out=v_b, in_=Vp[o0 : o0 + ext, :].rearrange("(c p) d -> p c d", p=128)
                )
                return kf_sb, v_b

            def poly_block(kf_sb, b, ln, jqs, emit_scores):
                """Clamped degree-9 odd tanh for several queries on DVE/Pool,
                step-interleaved across queries so the two in-order queues
                pipeline instead of serializing on one chain's latency."""
                if not jqs:
                    return
                c0_, c1_, c2_, c3_, c4_ = _PC
                n = len(jqs)
                A, V, U, Bw = [], [], [], []
                for i, jq in enumerate(jqs):
                    q = b * QPC + jq
                    a = tpoly.tile([128, ln], _F16, tag="tp", name=f"pa{i}")
                    nc.vector.tensor_scalar(
                        out=a, in0=kf_sb[:, 0:ln], scalar1=qfT_sb[:, q : q + 1],
                        scalar2=_C, op0=AL.add, op1=AL.min,
                    )
                    bal.add_dve(_ts4(ln))
                    A.append(a)
                for i in range(n):
                    v = tpoly.tile([128, ln], _F16, tag="tp", name=f"pv{i}")
                    flex_ts(v, A[i], -_C, None, AL.max, None, ln)
                    V.append(v)
                for i in range(n):
                    u = tpoly.tile([128, ln], _F16, tag="tp", name=f"pu{i}")
                    flex_tt(u, V[i], V[i], ln)
                    U.append(u)
                for i in range(n):
                    flex_ts(A[i], U[i], c4_, c3_, AL.mult, AL.add, ln)
                for i in range(n):
                    bw = tpoly.tile([128, ln], _F16, tag="tp", name=f"pb{i}")
                    flex_tt(bw, A[i], U[i], ln)
                    Bw.append(bw)
                for i in range(n):
                    flex_ts(A[i], Bw[i], c2_, None, AL.add, None, ln)
                for i in range(n):
                    flex_tt(Bw[i], A[i], U[i], ln)
                for i in range(n):
                    flex_ts(A[i], Bw[i], c1_, None, AL.add, None, ln)
                for i in range(n):
                    flex_tt(Bw[i], A[i], U[i], ln)
                for i in range(n):
                    flex_ts(A[i], Bw[i], c0_, None, AL.add, None, ln)
                for i in range(n):
                    to = tpoly.tile([128, ln], _F16, tag="to", name=f"pt{i}", bufs=10)
                    flex_tt(to, A[i], V[i], ln)
                    emit_scores(lambda kc, r, _t=to: _t[:, kc * 128 : kc * 128 + r], jqs[i])

            def make_sps(b):
                nkc, ln = nkcs[b], lens[b]
                s_ps = ps_s.tile([128, nkc, QPC], _F32, tag="s")
                rl = ln - 128 * (nkc - 1)
                if rl < 128:
                    base = 96 if rl >= 96 else (64 if rl >= 64 else 0)
                    nc.tensor.matmul(
                        out=s_ps[base:128, nkc - 1, :], lhsT=ones[:, 0 : 128 - base],
                        rhs=negq, start=True, stop=True,
                        skip_group_check=True, tile_position=(0, base),
                    )

                def emit_scores(src_ap_fn, q):
                    for kc in range(nkc):
                        r = min(128, ln - kc * 128)
                        nc.tensor.matmul(
                            out=s_ps[0:r, kc, q : q + 1],
                            lhsT=src_ap_fn(kc, r),
                            rhs=wv_sb,
                            start=True,
                            stop=True,
                        )

                return s_ps, emit_scores

            def scores_main(b, kf_sb, emit_scores, prefetch_cb=None):
                ln, y = lens[b], y_bs[b]
                na = QPC - y
                groups = []
                j = 0
                while j < na:
                    gsz = min(_G, na - j)
                    groups.append((j, gsz))
                    j += gsz

                def emit_adds(gi):
                    j0, gsz = groups[gi]
                    Xg = xg.tile([128, gsz, ln], _F16, tag="x")
                    for g in range(gsz):
                        q = b * QPC + j0 + g
                        nc.vector.tensor_scalar(
                            out=Xg[:, g, :], in0=kf_sb[:, 0:ln],
                            scalar1=qfT_sb[:, q : q + 1], scalar2=None, op0=AL.add,
                        )
                        bal.add_dve(_ts4(ln))
                    return Xg

                # ALL adds are issued ahead of the next batch's K-path copies
                # and this batch's poly chains on the DVE queue, so ACT's
                # tanh stream never waits on them.
                xtiles = [emit_adds(gi) for gi in range(len(groups))]
                if prefetch_cb is not None:
                    prefetch_cb()
                poly_block(kf_sb, b, ln, list(range(na, QPC)), emit_scores)
                for gi, (j0, gsz) in enumerate(groups):
                    Xg = xtiles[gi]
                    nc.scalar.activation(out=Xg, in_=Xg, func=AF.Tanh, bias=0.0, scale=1.0)
                    for g in range(gsz):
                        emit_scores(
                            lambda kc, r, _X=Xg, _g=g: _X[:, _g, kc * 128 : kc * 128 + r],
                            j0 + g,
                        )

            def epilogue(b, s_ps, v_b):
                nkc, ln = nkcs[b], lens[b]
                e_b = epool.tile([128, nkc, QPC], _F16, tag="e")
                nc.scalar.activation(out=e_b, in_=s_ps, func=AF.Exp, bias=0.0, scale=1.0)
                o_ps = ps_tail.tile([QPC, DV], _F32, tag="tail")
                rs_ps = ps_tail.tile([QPC, 1], _F32, tag="rs")
                for kc in range(nkc):
                    nc.tensor.matmul(
                        out=o_ps, lhsT=e_b[:, kc, :], rhs=v_b[:, kc, :],
                        start=(kc == 0), stop=(kc == nkc - 1),
                    )
                    nc.tensor.matmul(
                        out=rs_ps, lhsT=e_b[:, kc, :], rhs=onecol,
                        start=(kc == 0), stop=(kc == nkc - 1),
                    )
                rinv = opool.tile([QPC, 1], _F32, tag="ri")
                nc.vector.reciprocal(rinv, rs_ps)
                osb = opool.tile([QPC, DV], _F32, tag="o")
                nc.vector.tensor_scalar(
                    out=osb, in0=o_ps, scalar1=rinv, scalar2=None, op0=AL.mult
                )
                bal.add_dve(_cp1p(512))
                nc.sync.dma_start(out=outp[b, :, :], in_=osb)

            # ---- software-pipelined batch loop ---------------------------
            # k_path(b+1) is issued before scores(b) so the PE transposes and
            # DVE copies of the next batch aren't queued behind score matmuls
            # that wait on tanh; epilogue(b) is issued after scores(b+1).
            bo = _batch_order(lens)
            kvs = {bo[0]: k_path(bo[0])}
            q_path()
            pending = None
            for i, b in enumerate(bo):
                cb = None
                if i + 1 < B:
                    nxt = bo[i + 1]
                    cb = lambda _n=nxt: kvs.__setitem__(_n, k_path(_n))
                kf_sb, v_b = kvs.pop(b)
                s_ps, emit_sc = make_sps(b)
                scores_main(b, kf_sb, emit_sc, prefetch_cb=cb)
                epilogue(b, s_ps, v_b)

    nc.finalize()
    return nc


def _get_nc(lens):
    key = tuple(int(l) for l in lens)
    if key not in _cached:
        _cached[key] = _build(key)
    return _cached[key]


def kernel(Q, K, V, valid_lengths, W_q, W_k, w_v, _want_trace=False):
    Q = np.asarray(Q, dtype=np.float32)
    K = np.asarray(K, dtype=np.float32)
    V = np.asarray(V, dtype=np.float32)
    vl = np.asarray(valid_lengths).astype(np.int64).reshape(B)
    W_q = np.asarray(W_q, dtype=np.float32)
    W_k = np.asarray(W_k, dtype=np.float32)
    w_v = np.asarray(w_v, dtype=np.float32)

    lens = np.clip(vl, 1, LK)
    extents = np.clip(np.ceil(lens / 128.0).astype(int) * 128, 128, LK)
    nc = _get_nc(lens)

    f16 = np.float16
    Kc = np.concatenate([K[b, : extents[b], :] for b in range(B)], axis=0).astype(f16)
    Vc = np.concatenate([V[b, : extents[b], :] for b in range(B)], axis=0).astype(f16)
    Wqb = W_q.astype(f16)
    Wkb = W_k.astype(f16)
    wvb = w_v.reshape(H, 1).astype(f16)
    Qb = Q.astype(f16)

    in_maps = []
    for c in range(N_CORES):
        Qcore = np.concatenate(
            [Qb[b, c * QPC : (c + 1) * QPC, :] for b in range(B)], axis=0
        )
        in_maps.append(
            {
                "Q": Qcore,
                "K": Kc,
                "V": Vc,
                "Wq": Wqb,
                "Wk": Wkb,
                "wv": wvb,
                "ident": np.eye(128, dtype=f16),
            }
        )

    kwargs = {"trace": True} if _want_trace else {}
    res = run_bass_kernel_spmd(nc, in_maps, core_ids=list(range(N_CORES)), **kwargs)
    out = np.empty((B, LQ, DV), dtype=np.float32)
    for c in range(N_CORES):
        oc = res.results[c]["out"]  # (B, QPC, DV)
        for b in range(B):
            out[b, c * QPC : (c + 1) * QPC, :] = oc[b]
    if _want_trace:
        _cached["last_result"] = res
    return out
